# revision 3
# baseline (speedup 1.0000x reference)
"""MultiHeadAttention (B=2, S=2048, D=1024, H=16) on 8 trn2 cores.

Tensor-parallel over heads: core c owns heads 2c, 2c+1 (128 output features).

Per core:
  * Projections: W shards resident in SBUF; X^T streamed as host-repacked
    contiguous panels (8KB/partition runs). PSUM drained to bf16 on DVE so
    the scalar engine stays exp-only. v additionally PE-transposed into
    v_aug (token-major, with a ones column per 128-token chunk so the
    softmax denominator rides the attn@v matmul).
  * Attention per (batch, 512-query block, 128-key chunk): the two heads'
    scores matmuls are ROW-TILED — head0 as K=64 at tile_position (0,0),
    head1 at (64,0) — and execute concurrently on disjoint PE row groups
    into one fused [128,1024] PSUM tile. One exp (ACT) covers both heads;
    DVE multiplies in the not-mask; attn@v accumulates out^T[65, q] over
    key chunks in PSUM (row 64 = denominator). Division happens on host.
  * The not-mask for batch 0 travels as uint8 and is widened to bf16 on
    the otherwise-idle ACT during the projection prefix; batch 1's mask
    streams as bf16 during batch 0's attention, when the rings have slack.
  * Emission order is tuned so DMA ring bandwidth, PE, ACT and DVE stay
    overlapped: batch-1 projections and mask chunks trickle into filler
    slots inside batch-0's attention blocks.
  * PSUM budget (8 banks): proj/transpose 2, scores 2x2, attn@v 2x1.
"""

import sys

sys.path.insert(0, "/opt/trn_rl_repo")

import numpy as np
import ml_dtypes

import concourse.mybir as mybir
import concourse.tile as tile
from concourse import bacc
from concourse.bass_utils import run_bass_kernel_spmd
from concourse.masks import make_identity

BF16 = mybir.dt.bfloat16
F32 = mybir.dt.float32
NP_BF16 = ml_dtypes.bfloat16

NCORES = 8
B, S, D = 2, 2048, 1024
H, DH = 16, 64
HPC = H // NCORES  # heads per core = 2
MPC = HPC * DH  # output features per core = 128
T = B * S  # 4096 tokens
NKC = D // 128  # 8 contraction chunks for projections
NCH = T // 512  # 8 global 512-token chunks
QW = 512  # query-block width in attention
NQB = S // QW  # 4 q-blocks per batch
NJC = S // 128  # 16 key-token chunks per batch
NTC = T // 128  # 32 global token chunks (v_aug)
VW = DH + 1  # 65: head dim + ones column

_CACHE: dict = {}


def _emit(nc, dins, dout):
    from contextlib import ExitStack

    tc = dins["_tc"]
    with ExitStack() as ctx:
        singles = ctx.enter_context(tc.tile_pool(name="singles", bufs=1))

        w_sb = {}
        for t in ("q", "k", "v"):
            w = singles.tile([128, NKC, 128], BF16, tag=f"w{t}")
            nc.sync.dma_start(out=w, in_=dins[f"w{t}"].ap())
            w_sb[t] = w

        qT = singles.tile([128, T], BF16, tag="qT")
        kT = singles.tile([128, T], BF16, tag="kT")
        vT = singles.tile([128, T], BF16, tag="vT")

        v_aug = [
            singles.tile([128, NTC * VW], BF16, tag=f"vaug{h}", name=f"vaug{h}")
            for h in range(HPC)
        ]
        for h in range(HPC):
            # only the ones-column needs initializing; the 64 value columns
            # are fully written by the v transposes
            ones_col = v_aug[h].rearrange("p (c w) -> p c w", c=NTC)[:, :, DH : DH + 1]
            nc.vector.memset(ones_col, 1.0)

        ident = singles.tile([128, 128], BF16, tag="ident")
        make_identity(nc, ident)

        nm_sb = [
            singles.tile([128, NJC, S], BF16, tag=f"nm{b}", name=f"nm{b}")
            for b in range(B)
        ]


        xp = ctx.enter_context(tc.tile_pool(name="xpanels", bufs=3))
        etp = ctx.enter_context(tc.tile_pool(name="expp", bufs=4))
        outsb = ctx.enter_context(tc.tile_pool(name="outsb", bufs=2))
        nm8p = ctx.enter_context(tc.tile_pool(name="nm8", bufs=3))
        # PSUM (8 banks): proj/vt 2, scores 2x2, av-accum 2x1
        psA = ctx.enter_context(tc.tile_pool(name="psA", bufs=2, space="PSUM"))
        psS = ctx.enter_context(tc.tile_pool(name="psS", bufs=2, space="PSUM"))
        psO = ctx.enter_context(tc.tile_pool(name="psO", bufs=2, space="PSUM"))

        prefetched: dict = {}

        def prefetch_x(t, i):
            """Issue the DMA for a projection panel ahead of its matmuls."""
            xtile = xp.tile([128, NKC, 512], BF16, tag="xpanel", name="xpanel")
            nc.sync.dma_start(out=xtile, in_=dins[f"x{t}"].ap()[i])
            prefetched[(t, i)] = xtile

        def proj_chunk(t, i):
            """Project one global 512-token chunk of tensor t."""
            xtile = prefetched.pop((t, i), None)
            if xtile is None:
                xtile = xp.tile([128, NKC, 512], BF16, tag="xpanel", name="xpanel")
                nc.sync.dma_start(out=xtile, in_=dins[f"x{t}"].ap()[i])
            ps = psA.tile([128, 512], F32, tag="proj")
            for kc in range(NKC):
                nc.tensor.matmul(
                    ps,
                    lhsT=w_sb[t][:, kc, :],
                    rhs=xtile[:, kc, :],
                    start=(kc == 0),
                    stop=(kc == NKC - 1),
                )
            dst = {"q": qT, "k": kT, "v": vT}[t]
            # drain on DVE — ACT stays exp-only (GPSIMD cannot read PSUM)
            nc.vector.tensor_copy(out=dst[:, i * 512 : (i + 1) * 512], in_=ps)

        def v_transpose(i):
            """PE-transpose 512 projected v columns into v_aug (4 chunks)."""
            tbase = i * 4
            pst = psA.tile([128, 512], BF16, tag="proj", name="vtps")
            for j in range(4):
                nc.tensor.transpose(
                    out=pst[:, j * 128 : (j + 1) * 128],
                    in_=vT[:, (tbase + j) * 128 : (tbase + j + 1) * 128],
                    identity=ident,
                )
            for h in range(HPC):
                src = pst.rearrange("p (j d) -> p j d", j=4)[
                    :, :, h * DH : (h + 1) * DH
                ]
                dst = v_aug[h][:, tbase * VW : (tbase + 4) * VW].rearrange(
                    "p (j w) -> p j w", j=4
                )[:, :, 0:DH]
                nc.vector.tensor_copy(out=dst, in_=src)

        def emit_mask_chunk(b, jc):
            nc.sync.dma_start(
                out=nm_sb[b][:, jc, :],
                in_=dins["nmT"].ap()[b, jc * 128 : (jc + 1) * 128, :],
            )

        def emit_mask_chunk8(b, jc):
            """u8 over the wire; widen on ACT (idle during the prefix)."""
            nm8 = nm8p.tile([128, S], mybir.dt.uint8, tag="nm8")
            nc.sync.dma_start(
                out=nm8, in_=dins["nm8T"].ap()[b, jc * 128 : (jc + 1) * 128, :]
            )
            nc.scalar.copy(out=nm_sb[b][:, jc, :], in_=nm8)

        def emit_attn_block(b, qb, fillers=()):
            fillers = list(fillers)
            qcol = b * S + qb * QW
            psO_h = [
                psO.tile([VW, QW], F32, tag="av", name=f"av{h}") for h in range(HPC)
            ]
            pend = []  # deferred av matmuls (prev jc) to keep PE un-stalled
            for jc in range(NJC):
                if jc % 2 == 1 and fillers:
                    fillers.pop(0)()
                if jc == NJC - 1:
                    while fillers:
                        fillers.pop(0)()
                tglob = b * NJC + jc
                for mm in pend[:2]:
                    nc.tensor.matmul(**mm)
                pend = pend[2:]
                ps = psS.tile([128, 2 * QW], F32, tag="scores")
                for h in range(HPC):
                    nc.tensor.matmul(
                        ps[:, h * QW : (h + 1) * QW],
                        lhsT=kT[h * DH : (h + 1) * DH, tglob * 128 : (tglob + 1) * 128],
                        rhs=qT[h * DH : (h + 1) * DH, qcol : qcol + QW],
                        start=True,
                        stop=True,
                        tile_position=(h * DH, 0),
                    )
                et = etp.tile([128, 2 * QW], BF16, tag="exp")
                nc.scalar.activation(
                    out=et, in_=ps, func=mybir.ActivationFunctionType.Exp
                )
                for h in range(HPC):
                    nc.vector.tensor_mul(
                        et[:, h * QW : (h + 1) * QW],
                        et[:, h * QW : (h + 1) * QW],
                        nm_sb[b][:, jc, qb * QW : (qb + 1) * QW],
                    )
                for h in range(HPC):
                    pend.append(
                        dict(
                            out=psO_h[h],
                            lhsT=v_aug[h][:, tglob * VW : tglob * VW + VW],
                            rhs=et[:, h * QW : (h + 1) * QW],
                            start=(jc == 0),
                            stop=(jc == NJC - 1),
                        )
                    )
            for mm in pend:
                nc.tensor.matmul(**mm)
            for h in range(HPC):
                osb = outsb.tile([VW, QW], BF16, tag="osb")
                nc.vector.tensor_copy(out=osb, in_=psO_h[h])
                nc.sync.dma_start(
                    out=dout.ap()[b, h, :, qb * QW : (qb + 1) * QW], in_=osb
                )

        def mk(f, *a):
            return lambda: f(*a)

        for _ in range(dins.get("_repeat", 1)):
            # batch-0 prefix, ring-order balanced so v_aug/qT/kT all complete
            # about when the first attention block can start; nm(0) rides as
            # u8 interleaved with the panels and widens on the idle ACT
            proj_chunk("v", 0)
            emit_mask_chunk8(0, 0)
            emit_mask_chunk8(0, 1)
            v_transpose(0)
            proj_chunk("q", 0)
            emit_mask_chunk8(0, 2)
            emit_mask_chunk8(0, 3)
            proj_chunk("k", 0)
            for i in range(1, 4):
                proj_chunk("v", i)
                emit_mask_chunk8(0, 4 * i)
                emit_mask_chunk8(0, 4 * i + 1)
                v_transpose(i)
                emit_mask_chunk8(0, 4 * i + 2)
                emit_mask_chunk8(0, 4 * i + 3)
                proj_chunk("k", i)
            prefetch_x("q", 1)
            prefetch_x("q", 2)
            # batch-0 attention; b0 q-chunks + all of batch-1 prep trickled in
            fillers = [
                [mk(proj_chunk, "q", 1)],
                [mk(proj_chunk, "q", 2), mk(prefetch_x, "q", 3),
                 mk(prefetch_x, "k", 4), mk(proj_chunk, "k", 4),
                 mk(prefetch_x, "v", 4), mk(proj_chunk, "v", 4),
                 mk(v_transpose, 4), mk(prefetch_x, "k", 5)],
                [mk(proj_chunk, "q", 3), mk(proj_chunk, "k", 5),
                 mk(prefetch_x, "v", 5), mk(proj_chunk, "v", 5),
                 mk(v_transpose, 5), mk(prefetch_x, "k", 6),
                 mk(proj_chunk, "k", 6), mk(prefetch_x, "q", 4)],
                [mk(proj_chunk, "q", 4), mk(prefetch_x, "v", 6),
                 mk(proj_chunk, "v", 6), mk(v_transpose, 6),
                 mk(prefetch_x, "k", 7), mk(proj_chunk, "k", 7),
                 mk(prefetch_x, "v", 7), mk(proj_chunk, "v", 7),
                 mk(v_transpose, 7)]
                + [mk(emit_mask_chunk, 1, jc) for jc in range(NJC)],
            ]
            for qb in range(NQB):
                emit_attn_block(0, qb, fillers[qb])
            # batch-1 attention; its remaining q chunks trickled in
            fillers1 = [
                [mk(prefetch_x, "q", 5), mk(proj_chunk, "q", 5)],
                [mk(prefetch_x, "q", 6), mk(proj_chunk, "q", 6)],
                [mk(prefetch_x, "q", 7), mk(proj_chunk, "q", 7)],
                [],
            ]
            for qb in range(NQB):
                emit_attn_block(1, qb, fillers1[qb])


def _build(repeat=1):
    key = ("nc2", repeat)
    if key in _CACHE:
        return _CACHE[key]
    nc = bacc.Bacc("TRN2", target_bir_lowering=False, debug=False)
    dins = {}
    for t in ("q", "k", "v"):
        dins[f"x{t}"] = nc.dram_tensor(f"x{t}", [NCH, 128, NKC, 512], BF16,
                                       kind="ExternalInput")
        dins[f"w{t}"] = nc.dram_tensor(f"w{t}", [128, NKC, 128], BF16,
                                       kind="ExternalInput")
    dins["nmT"] = nc.dram_tensor("nmT", [B, S, S], BF16, kind="ExternalInput")
    dins["nm8T"] = nc.dram_tensor("nm8T", [B, S, S], mybir.dt.uint8,
                                  kind="ExternalInput")
    dout = nc.dram_tensor("out", [B, HPC, VW, S], BF16, kind="ExternalOutput")

    with tile.TileContext(nc) as tc:
        dins["_tc"] = tc
        dins["_repeat"] = repeat
        _emit(nc, dins, dout)
        del dins["_tc"], dins["_repeat"]
    nc.compile()
    _CACHE[key] = nc
    return nc


def _prep_inputs(query, key, value, mask, Wq, bq, Wk, bk, Wv, bv):
    """Host-side shard prep. Returns per-core input maps."""
    for b in (bq, bk, bv):
        assert not np.any(np.asarray(b)), "nonzero projection bias unsupported"

    xs = {}
    for name, x in (("q", query), ("k", key), ("v", value)):
        # xrep[i, p, c, n] = X[i*512+n, c*128+p]
        xt = (
            np.asarray(x, dtype=np.float32)
            .reshape(NCH, 512, NKC, 128)
            .transpose(0, 3, 2, 1)
        )
        xs[f"x{name}"] = np.ascontiguousarray(xt).astype(NP_BF16)

    nm8 = (~np.asarray(mask)).astype(np.uint8)
    nm8T = np.ascontiguousarray(np.transpose(nm8, (0, 2, 1)))
    nmT = nm8T.astype(NP_BF16)

    Wq = np.asarray(Wq, dtype=np.float32)
    Wk = np.asarray(Wk, dtype=np.float32)
    Wv = np.asarray(Wv, dtype=np.float32)
    scale = 1.0 / np.sqrt(np.float32(DH))

    in_maps = []
    for c in range(NCORES):
        r = slice(c * MPC, (c + 1) * MPC)
        m = dict(xs)
        m["nmT"] = nmT

        def wrep(Wsh):
            # w[p, c, m] = Wsh[m, c*128+p]
            w = Wsh.reshape(MPC, NKC, 128).transpose(2, 1, 0)
            return np.ascontiguousarray(w).astype(NP_BF16)

        m["wq"] = wrep(Wq[r] * scale)
        m["wk"] = wrep(Wk[r])
        m["wv"] = wrep(Wv[r])
        in_maps.append(m)
    return in_maps


def _assemble(results):
    """results: per-core dicts with 'out' [B, HPC, 65, S] f32 -> [B, S, D]."""
    full = np.empty((B, S, D), dtype=np.float32)
    for c in range(NCORES):
        o = results[c]["out"]
        for b in range(B):
            for h in range(HPC):
                num = o[b, h, :DH, :].astype(np.float32)  # [64, S]
                den = o[b, h, DH, :].astype(np.float32)  # [S]
                col = c * MPC + h * DH
                full[b, :, col : col + DH] = (num / den).T
    return full


def kernel(query, key, value, mask, Wq, bq, Wk, bk, Wv, bv, **extra):
    nc = _build()
    in_maps = _prep_inputs(query, key, value, mask, Wq, bq, Wk, bk, Wv, bv)
    res = run_bass_kernel_spmd(nc, in_maps, core_ids=list(range(NCORES)))
    return _assemble(res.results)


def run_traced(inputs, **trace_kwargs):
    """For test.py: run with NTFF tracing, return (output, BassKernelResults)."""
    nc = _build()
    in_maps = _prep_inputs(**{k: inputs[k] for k in (
        "query", "key", "value", "mask", "Wq", "bq", "Wk", "bk", "Wv", "bv")})
    res = run_bass_kernel_spmd(
        nc, in_maps, core_ids=list(range(NCORES)), trace=True, **trace_kwargs
    )
    return _assemble(res.results), res
